# revision 80
# baseline (speedup 1.0000x reference)
"""Trainium2 Bass kernel for nn_FFWRelativeCrossAttentionModule.

Sharding: 8 cores = (batch b in 0..3) x (query half qh in 0..1);
communication-free (attention only mixes query<->kv tokens).

v2 design notes (vs the v1 baseline):
- The K/V projection + K-rotary for layer i+1 is layer-independent, so it
  is emitted INSIDE layer i's attention pair loop to fill the PE idle
  gaps while ACT grinds through the softmax Exp (keeps the HAM clock
  warm; Exp on ACT is the hard floor of this kernel).
- K/V projections run in fp8e4m3 with MatmulPerfMode.DoubleRow
  (256-feature contraction per matmul): weights are host-packed
  [128,2sub,2kc,C] at x16 scale; the x1/16 is folded into the host-side
  ck/sk rotary codes (K) and the PSUM eviction scale (V).
- Elementwise work is spread across engines: ACT takes adaLN, ReLU and
  the LN copy/square (all in the pinned natural_log_exp_and_others
  table set); Pool (gpsimd) takes PSUM evictions / second half of the
  rotary; DVE keeps the shuffles and the fp32 residual path.
- 1/denominator is applied after out_proj (per-token scalar commutes
  through the feature contraction), removing the recip chain from the
  attention->oproj latency path.
- ck/sk/cq/sq stay resident in SBUF; weights are double-buffered so
  layer i+1's DMA overlaps layer i compute.
"""

import os
import sys

for _p in ("/opt/trn_rl_repo", "/root/.axon_site/_ro/trn_rl_repo"):
    if os.path.isdir(_p) and _p not in sys.path:
        sys.path.append(_p)

import numpy as np
import ml_dtypes

L, C, H, HD = 4, 512, 8, 64
NT = 512          # query tokens per core
NK = 2048         # kv tokens
CH = C // 128     # 4 chunks of 128 channels
SCALING = HD ** -0.5
W8SCALE = 16.0    # fp8 weight prescale
BF = ml_dtypes.bfloat16

_CACHE = {}


def _rot2_rows(w):
    # rot2(x)[2i] = -x[2i+1]; rot2(x)[2i+1] = x[2i], applied to the
    # projection output channels = rows of w.
    w2 = np.empty_like(w)
    w2[0::2] = -w[1::2]
    w2[1::2] = w[0::2]
    return w2


def _pack_w(m):
    # [C_in, C_out] -> [128, CH(kc), C_out] lhsT tile layout (bf16 weights)
    return np.ascontiguousarray(np.transpose(m.reshape(CH, 128, C), (1, 0, 2)))


def _pack_w8(m):
    # [C_in, C_out] -> [128, 2sub, 2kc, C_out] DoubleRow lhsT layout:
    # contraction feature f = kc*256 + sub*128 + p
    return np.ascontiguousarray(
        np.transpose(m.reshape(2, 2, 128, C), (2, 1, 0, 3)))


def _pack_fm(m):
    # feature-major [C, N] -> [128, CH, N]
    n = m.shape[1]
    return np.ascontiguousarray(np.transpose(m.reshape(CH, 128, n), (1, 0, 2)))


def _pack_v8(v):
    # token-major value [Nkv, C] -> [128, 2sub, 2kc, Nkv] with
    # vT8[p, s, c, t] = v[t, c*256 + s*128 + p]
    return np.ascontiguousarray(np.transpose(v.reshape(NK, 2, 2, 128), (3, 2, 1, 0)))


def _silu(x):
    return x / (1.0 + np.exp(-x))


def _pin_act_tables():
    """Make every activation resolve to natural_log_exp_and_others so the
    kernel uses one ACT table set (no ~1.3us reloads between funcs)."""
    from concourse import bacc as _bacc
    from concourse.hw_specs import get_activation_tables as _orig

    def patched(arch):
        tabs = _orig(arch)
        keep = "natural_log_exp_and_others"
        if keep in tabs:
            tabs = {k: (v if k == keep else set()) for k, v in tabs.items()}
        return tabs

    _bacc.get_activation_tables = patched


def _build(flags, nrep=1, unroll=1):
    import concourse.bass as bass
    import concourse.mybir as mybir
    import concourse.tile as tile
    from concourse import bacc

    _pin_act_tables()

    dt = mybir.dt
    AF = mybir.ActivationFunctionType
    AO = mybir.AluOpType
    DR = mybir.MatmulPerfMode.DoubleRow

    if any(flags):
        raise NotImplementedError("bias/ln-affine variants not supported")

    nc = bacc.Bacc("TRN2", target_bir_lowering=False, debug=False, num_devices=8)

    d_qT = nc.declare_dram_parameter("qT", [128, CH, NT], dt.bfloat16, isOutput=False)
    d_vT8 = nc.declare_dram_parameter("vT8", [128, 2, 2, NK], dt.float8e4, isOutput=False)
    d_cq = nc.declare_dram_parameter("cq", [128, CH, NT], dt.bfloat16, isOutput=False)
    d_sq = nc.declare_dram_parameter("sq", [128, CH, NT], dt.bfloat16, isOutput=False)
    d_ck = nc.declare_dram_parameter("ck", [128, CH, NK], dt.bfloat16, isOutput=False)
    d_sk = nc.declare_dram_parameter("sk", [128, CH, NK], dt.bfloat16, isOutput=False)
    d_w8 = nc.declare_dram_parameter("w8", [L, 3, 128, 2, 2, C], dt.float8e4, isOutput=False)
    d_wts = nc.declare_dram_parameter("wts", [L, 4, 128, CH, C], dt.bfloat16, isOutput=False)
    d_ada = nc.declare_dram_parameter("ada", [128, L * 4 * CH], dt.float32, isOutput=False)
    d_ind8 = nc.declare_dram_parameter("ind8", [8, C], dt.float32r, isOutput=False)
    d_ones1 = nc.declare_dram_parameter("ones1", [1, 128], dt.float32r, isOutput=False)
    d_out = nc.declare_dram_parameter("out", [L, 128, CH, NT], dt.bfloat16, isOutput=True)

    SWAP_MASK = [j + 1 if j % 2 == 0 else j - 1 for j in range(32)]
    WQ_NAMES = ["wq", "wo", "w1", "w2"]

    def ada_col(i, qty, c):
        return (i * 4 + qty) * CH + c

    with tile.TileContext(nc) as tc:
        with tc.tile_pool(name="const", bufs=1) as cpool, \
             tc.tile_pool(name="wkv", bufs=2) as kvpool, \
             tc.tile_pool(name="wq", bufs=2) as wqpool, \
             tc.tile_pool(name="state", bufs=1) as spool, \
             tc.tile_pool(name="epool", bufs=6) as epool, \
             tc.tile_pool(name="act", bufs=3) as apool, \
             tc.tile_pool(name="resid", bufs=2) as rpool, \
             tc.tile_pool(name="scr", bufs=4) as scrpool, \
             tc.tile_pool(name="scrf", bufs=2) as scfpool, \
             tc.tile_pool(name="small", bufs=2) as smpool, \
             tc.tile_pool(name="stats", bufs=3) as stpool, \
             tc.tile_pool(name="psum", bufs=2, space="PSUM") as ppool:

            # ---- resident constants; only the tensors the first PE work
            # needs are DMA'd here, the rest are issued inside body() so the
            # DMA queues aren't head-of-line blocked at kernel start ----
            ada = cpool.tile([128, L * 4 * CH], dt.float32)
            nc.sync.dma_start(ada[:], d_ada[:])
            vT8 = cpool.tile([128, 2, 2, NK], dt.float8e4)
            nc.sync.dma_start(vT8[:], d_vT8[:])
            cq = cpool.tile([128, CH, NT], dt.bfloat16)
            sq = cpool.tile([128, CH, NT], dt.bfloat16)
            ck = cpool.tile([128, CH, NK], dt.bfloat16)
            sk = cpool.tile([128, CH, NK], dt.bfloat16)
            indA = cpool.tile([6, C], dt.float32r)
            indB = cpool.tile([2, C], dt.float32r)
            ones_sc = cpool.tile([128, 1], dt.bfloat16)   # 1/C for mean matmuls
            nc.vector.memset(ones_sc[:], 1.0 / C)
            ones1 = cpool.tile([1, 128], dt.float32r)      # broadcast lhsT
            nc.sync.dma_start(ones1[:], d_ones1[:])

            # ---- persistent state ----
            q = spool.tile([128, CH, NT], dt.bfloat16)    # master q (feature-major)
            nc.sync.dma_start(q[:], d_qT[:])
            vb = spool.tile([128, NK // 128, H, HD + 1], dt.bfloat16)
            nc.vector.memset(vb[:, :, :, HD:HD + 1], 1.0)

            def load_late_consts():
                nc.sync.dma_start(cq[:], d_cq[:])
                nc.sync.dma_start(sq[:], d_sq[:])
                for _c in range(CH):
                    eng = nc.gpsimd if _c % 2 == 0 else nc.sync
                    eng.dma_start(ck[:, _c, :], d_ck[:, _c, :])
                    eng.dma_start(sk[:, _c, :], d_sk[:, _c, :])
                nc.sync.dma_start(indA[:], d_ind8[0:6, :])
                nc.sync.dma_start(indB[:], d_ind8[6:8, :])

            wkv = {}   # (layer, name) -> fp8 DoubleRow weight tile
            wq = {}    # (layer, name) -> bf16 weight tile
            kbs = {}   # layer -> rotary-embedded K.T tile

            def load_wkv(i):
                for j, nm in enumerate(["wk", "wk2", "wv"]):
                    t = kvpool.tile([128, 2, 2, C], dt.float8e4, tag=nm,
                                    name=f"{nm}8_{i}")
                    nc.sync.dma_start(t[:], d_w8[i, j])
                    wkv[(i, nm)] = t

            def load_wq(i):
                for j, nm in enumerate(WQ_NAMES):
                    t = wqpool.tile([128, CH, C], dt.bfloat16, tag=nm,
                                    name=f"{nm}_{i}")
                    nc.sync.dma_start(t[:], d_wts[i, j])
                    wq[(i, nm)] = t

            def kv_k_chunk(i, mc):
                # kproj chunk mc (fp8 DoubleRow, rot2 companion as a second
                # matmul) + rotary -> kbs[i][:, mc, :]
                kb = kbs[i]
                wk = wkv[(i, "wk")]
                wk2 = wkv[(i, "wk2")]
                for n in range(CH):
                    ksl = slice(n * 512, (n + 1) * 512)
                    pk = ppool.tile([128, 512], dt.float32, tag="kvmm",
                                    name=f"pk_{i}_{mc}_{n}")
                    for c in range(2):
                        nc.tensor.matmul(pk[:], wk[:, :, c, mc * 128:(mc + 1) * 128],
                                         vT8[:, :, c, ksl], start=(c == 0),
                                         stop=(c == 1), perf_mode=DR)
                    pk2 = ppool.tile([128, 512], dt.float32, tag="kvmm",
                                     name=f"pk2_{i}_{mc}_{n}")
                    for c in range(2):
                        nc.tensor.matmul(pk2[:], wk2[:, :, c, mc * 128:(mc + 1) * 128],
                                         vT8[:, :, c, ksl], start=(c == 0),
                                         stop=(c == 1), perf_mode=DR)
                    t1 = scrpool.tile([128, 512], dt.bfloat16, tag="scr",
                                      name=f"kc_{i}_{mc}_{n}")
                    nc.vector.scalar_tensor_tensor(
                        t1[:], pk[:], 0.0, ck[:, mc, ksl], AO.add, AO.mult)
                    t2 = scrpool.tile([128, 512], dt.bfloat16, tag="scr",
                                      name=f"ks_{i}_{mc}_{n}")
                    nc.vector.scalar_tensor_tensor(
                        t2[:], pk2[:], 0.0, sk[:, mc, ksl], AO.add, AO.mult)
                    nc.gpsimd.tensor_tensor(kb[:, mc, ksl], t1[:], t2[:], AO.add)

            def kv_v(i, idxs):
                # vproj (fp8 DoubleRow, token-major) -> vb ; x1/16 on evict.
                # Evictions stay off ACT so they never queue-block the LN
                # Ln/Exp chain.
                wv = wkv[(i, "wv")]
                for idx in idxs:
                    pv = ppool.tile([128, 512], dt.float32, tag="kvmm",
                                    name=f"pv_{i}_{idx}")
                    for c in range(2):
                        nc.tensor.matmul(pv[:], vT8[:, :, c, idx * 128:(idx + 1) * 128],
                                         wv[:, :, c, :], start=(c == 0),
                                         stop=(c == 1), perf_mode=DR)
                    if idx % 2 == 0:
                        nc.vector.tensor_scalar_mul(
                            vb[:, idx, :, 0:HD],
                            pv[:].rearrange("p (h d) -> p h d", h=H), 1.0 / W8SCALE)
                    else:
                        nc.scalar.mul(
                            vb[:, idx, :, 0:HD],
                            pv[:].rearrange("p (h d) -> p h d", h=H), 1.0 / W8SCALE)

            def layer(i):
                if i + 1 < L:
                    load_wq(i + 1)
                if i + 2 < L:
                    load_wkv(i + 2)

                # ---- adaln (attn) on DVE (4x mode: all-bf16 SBUF) ----
                xa = apool.tile([128, CH, NT], dt.bfloat16, tag="act", name=f"xa_{i}")
                for c in range(CH):
                    nc.vector.tensor_scalar(
                        xa[:, c, :], q[:, c, :],
                        ada[:, ada_col(i, 0, c):ada_col(i, 0, c) + 1],
                        ada[:, ada_col(i, 1, c):ada_col(i, 1, c) + 1],
                        AO.mult, AO.add)

                # ---- q projection + rotary -> qb ----
                qb = spool.tile([128, CH, NT], dt.bfloat16, tag="qb", name=f"qb_{i}")
                for mc in range(CH):
                    pq = ppool.tile([128, 512], dt.float32, tag="kvmm",
                                    name=f"pq_{i}_{mc}")
                    for kc in range(CH):
                        nc.tensor.matmul(pq[:], wq[(i, "wq")][:, kc, mc * 128:(mc + 1) * 128],
                                         xa[:, kc, :], start=(kc == 0), stop=(kc == CH - 1))
                    t1 = scrpool.tile([128, 512], dt.bfloat16, tag="scr",
                                      name=f"qc_{i}_{mc}")
                    nc.vector.scalar_tensor_tensor(
                        t1[:], pq[:], 0.0, cq[:, mc, :], AO.add, AO.mult)
                    qsh = scfpool.tile([128, 512], dt.float32, tag="ksh",
                                       name=f"qsh_{i}_{mc}")
                    nc.vector.stream_shuffle(qsh[:], pq[:], SWAP_MASK)
                    t2 = scrpool.tile([128, 512], dt.bfloat16, tag="scr",
                                      name=f"qs_{i}_{mc}")
                    nc.vector.scalar_tensor_tensor(
                        t2[:], qsh[:], 0.0, sq[:, mc, :], AO.add, AO.mult)
                    nc.gpsimd.tensor_tensor(qb[:, mc, :], t1[:], t2[:], AO.add)

                # ---- attention (+ next layer's K-proj interleaved) ----
                kb = kbs[i]
                o_raw = apool.tile([128, CH, NT], dt.bfloat16, tag="act",
                                   name=f"oraw_{i}")
                # pairs 0-2 and pair 3 get separate base-0 den/recip tiles:
                # group A's 1/den computes mid-attention, group B right at the
                # end — both ready before their rb matmuls in the post phase
                den2t = [smpool.tile([6, 512], dt.float32r, tag="den0", bufs=1,
                                     name=f"den0_{i}"),
                         smpool.tile([2, 512], dt.float32r, tag="den1", bufs=1,
                                     name=f"den1_{i}")]
                for p in range(H // 2):
                    hpair = (2 * p, 2 * p + 1)
                    if i == 0:
                        kv_k_chunk(0, p)
                    oacc = {h: ppool.tile([128, 512], dt.float32, tag="mm",
                                          name=f"oacc_{i}_{h}") for h in hpair}
                    Eg = {}

                    def attn_v(g):
                        for h in hpair:
                            for j in range(2):
                                nidx = g * 2 + j
                                nc.tensor.matmul(
                                    oacc[h][0:HD + 1, :], vb[:, nidx, h, :],
                                    Eg[(h, g)][:, j, :], start=(nidx == 0),
                                    stop=(nidx == 15), skip_group_check=True)

                    for g in range(8):
                        ps = {h: ppool.tile([128, 2, 512], dt.float32, tag="sc",
                                            name=f"ps_{i}_{h}_{g}") for h in hpair}
                        for j in range(2):
                            nidx = g * 2 + j
                            for h in hpair:
                                off = (h % 2) * 64
                                nc.tensor.matmul(
                                    ps[h][:, j, :],
                                    kb[off:off + 64, p, nidx * 128:(nidx + 1) * 128],
                                    qb[off:off + 64, p, :], start=True, stop=True)
                        for h in hpair:
                            E = epool.tile([128, 2, 512], dt.bfloat16, tag="E",
                                           name=f"E_{i}_{h}_{g}")
                            nc.scalar.activation(E[:], ps[h][:], AF.Exp)
                            Eg[(h, g)] = E
                        if g >= 2:
                            attn_v(g - 2)
                    attn_v(6)
                    attn_v(7)
                    half = 0 if p < 3 else 1
                    for h in hpair:
                        off = (h % 2) * 64
                        nc.vector.tensor_copy(o_raw[off:off + 64, p, :], oacc[h][0:HD, :])
                        dh = smpool.tile([1, 512], dt.float32r, tag="denh", bufs=1,
                                         name=f"dh_{i}_{h}")
                        nc.vector.tensor_copy(dh[:], oacc[h][HD:HD + 1, :])
                        row = (2 * p + h % 2) if p < 3 else (h % 2)
                        nc.sync.dma_start(den2t[half][row:row + 1, :], dh[:])
                    if p == 2 or p == 3:
                        # 1/den for this group, in place, inside the exp
                        # window (off the post critical path)
                        nc.scalar.activation(den2t[half][:], den2t[half][:], AF.Ln)
                        nc.scalar.activation(den2t[half][:], den2t[half][:],
                                             AF.Exp, scale=-1.0)
                    if i + 1 < L:
                        kv_k_chunk(i + 1, p)

                # ---- out projection; r1 = po*recip + q ; fused LN1 stats.
                # rb/finish lag one chunk behind po so the PE queue never
                # head-of-line blocks on the recip chain or evictions.
                r1 = rpool.tile([128, CH, NT], dt.float32, tag="resid", name=f"r1_{i}")
                pm1 = ppool.tile([128, 512], dt.float32, tag="mm", name=f"pm1_{i}")
                pv1 = ppool.tile([128, 512], dt.float32, tag="mm", name=f"pv1_{i}")
                store1 = {}
                po = {}

                def fin1(mo):
                    ind = indA if mo < 3 else indB
                    rb = ppool.tile([128, 512], dt.float32, tag="sc", name=f"rb_{i}_{mo}")
                    nc.tensor.matmul(rb[:], ind[:, mo * 128:(mo + 1) * 128],
                                     den2t[0 if mo < 3 else 1][:], start=True, stop=True)
                    rb_s = scrpool.tile([128, 512], dt.float32, tag="rbs", bufs=2,
                                        name=f"rbs_{i}_{mo}")
                    nc.scalar.copy(rb_s[:], rb[:])
                    t = scfpool.tile([128, 512], dt.float32, tag="ksh",
                                     name=f"rt_{i}_{mo}")
                    nc.vector.tensor_mul(t[:], po[mo][:], rb_s[:])
                    nc.gpsimd.tensor_tensor(r1[:, mo, :], t[:], q[:, mo, :], AO.add)
                    _ln_chunk_evict(i, 0, r1, mo, store1)
                    if mo >= 1:
                        _ln_stats_mm(store1, mo - 1, pm1, pv1)

                for mo in range(CH):
                    po[mo] = ppool.tile([128, 512], dt.float32, tag="kvmm",
                                        name=f"po_{i}_{mo}")
                    for kc in range(CH):
                        nc.tensor.matmul(po[mo][:], wq[(i, "wo")][:, kc, mo * 128:(mo + 1) * 128],
                                         o_raw[:, kc, :], start=(kc == 0), stop=(kc == CH - 1))
                    if mo >= 1:
                        fin1(mo - 1)
                fin1(CH - 1)
                _ln_stats_mm(store1, CH - 1, pm1, pv1)
                ln1 = _ln_head(i, 0, pm1, pv1)

                # ---- next layer's V projection, first half (PE slack while
                # the LN1 stats chain resolves) ----
                if i + 1 < L:
                    kv_v(i + 1, range(0, 8))

                # ---- LN1 -> q, with per-chunk adaln(ffn) follow-on ----
                xfb = apool.tile([128, CH, NT], dt.bfloat16, tag="act", name=f"xfb_{i}")

                def emit_xfb(c):
                    nc.vector.tensor_scalar(
                        xfb[:, c, :], q[:, c, :],
                        ada[:, ada_col(i, 2, c):ada_col(i, 2, c) + 1],
                        ada[:, ada_col(i, 3, c):ada_col(i, 3, c) + 1],
                        AO.mult, AO.add)

                _ln_apply(i, 0, r1, *ln1, after_chunk=emit_xfb)

                # ---- FFN ; fused LN2 stats ----
                hbf = apool.tile([128, CH, NT], dt.bfloat16, tag="act", name=f"hbf_{i}")
                for mh in range(CH):
                    ph = ppool.tile([128, 512], dt.float32, tag="kvmm", name=f"ph_{i}_{mh}")
                    for kc in range(CH):
                        nc.tensor.matmul(ph[:], wq[(i, "w1")][:, kc, mh * 128:(mh + 1) * 128],
                                         xfb[:, kc, :], start=(kc == 0), stop=(kc == CH - 1))
                    nc.vector.tensor_scalar_max(hbf[:, mh, :], ph[:], 0.0)
                r2 = rpool.tile([128, CH, NT], dt.float32, tag="resid", name=f"r2_{i}")
                pm2 = ppool.tile([128, 512], dt.float32, tag="mm", name=f"pm2_{i}")
                pv2 = ppool.tile([128, 512], dt.float32, tag="mm", name=f"pv2_{i}")
                store2 = {}
                pf = {}

                def fin2(mo):
                    nc.vector.tensor_add(r2[:, mo, :], pf[mo][:], xfb[:, mo, :])
                    _ln_chunk_evict(i, 1, r2, mo, store2)
                    if mo >= 1:
                        _ln_stats_mm(store2, mo - 1, pm2, pv2)

                for mo in range(CH):
                    pf[mo] = ppool.tile([128, 512], dt.float32, tag="kvmm",
                                        name=f"pf_{i}_{mo}")
                    for kc in range(CH):
                        nc.tensor.matmul(pf[mo][:], wq[(i, "w2")][:, kc, mo * 128:(mo + 1) * 128],
                                         hbf[:, kc, :], start=(kc == 0), stop=(kc == CH - 1))
                    if mo >= 1:
                        fin2(mo - 1)
                fin2(CH - 1)
                _ln_stats_mm(store2, CH - 1, pm2, pv2)
                ln2 = _ln_head(i, 1, pm2, pv2)

                # ---- next layer's V projection, second half (PE slack while
                # the LN2 stats chain resolves) ----
                if i + 1 < L:
                    kv_v(i + 1, range(8, 16))

                # ---- LN2 -> q ; emit layer output per chunk ----
                _ln_apply(i, 1, r2, *ln2,
                          after_chunk=lambda c: nc.sync.dma_start(d_out[i, :, c, :], q[:, c, :]))

            def _ln_chunk_evict(i, which, rin, c, store):
                # bf16 copy + square right after rin chunk lands (DVE + Pool)
                rbf = scrpool.tile([128, 512], dt.bfloat16, tag="scr",
                                   name=f"rbf_{i}_{which}_{c}")
                nc.vector.tensor_copy(rbf[:], rin[:, c, :])
                r2b = scrpool.tile([128, 512], dt.bfloat16, tag="scr",
                                   name=f"r2b_{i}_{which}_{c}")
                nc.vector.tensor_mul(r2b[:], rbf[:], rbf[:])
                store[c] = (rbf, r2b)

            def _ln_stats_mm(store, c, pm, pv):
                # lag-1 accumulation so the PE queue rarely waits on evicts
                rbf, r2b = store[c]
                nc.tensor.matmul(pm[0:1, :], ones_sc[:], rbf[:],
                                 start=(c == 0), stop=(c == CH - 1),
                                 skip_group_check=True)
                nc.tensor.matmul(pv[0:1, :], ones_sc[:], r2b[:],
                                 start=(c == 0), stop=(c == CH - 1),
                                 skip_group_check=True)

            def _ln_head(i, which, pm, pv):
                # stat chain emitted FIRST so the DVE/ACT queue heads aren't
                # blocked behind filler work
                m_sb = stpool.tile([1, 512], dt.float32r, tag="st", name=f"m_{i}_{which}")
                nc.vector.tensor_copy(m_sb[:], pm[0:1, :])
                msq = stpool.tile([1, 512], dt.float32, tag="st", name=f"msq_{i}_{which}")
                nc.scalar.activation(msq[:], pm[0:1, :], AF.Square)
                var = stpool.tile([1, 512], dt.float32, tag="st", name=f"var_{i}_{which}")
                nc.vector.scalar_tensor_tensor(var[:], pv[0:1, :], 1e-5, msq[:],
                                               AO.add, AO.subtract)
                lnv = stpool.tile([1, 512], dt.float32, tag="st", name=f"lnv_{i}_{which}")
                nc.scalar.activation(lnv[:], var[:], AF.Ln)
                rstd = stpool.tile([1, 512], dt.float32r, tag="st", name=f"rstd_{i}_{which}")
                nc.scalar.activation(rstd[:], lnv[:], AF.Exp, scale=-0.5)
                return m_sb, rstd

            def _ln_apply(i, which, rin, m_sb, rstd, after_chunk=None):
                mb = ppool.tile([128, 512], dt.float32, tag="sc", name=f"mb_{i}_{which}")
                nc.tensor.matmul(mb[:], ones1[:], m_sb[:], start=True, stop=True)
                rsb = ppool.tile([128, 512], dt.float32, tag="sc", name=f"rsb_{i}_{which}")
                nc.tensor.matmul(rsb[:], ones1[:], rstd[:], start=True, stop=True)
                for c in range(CH):
                    t1 = scfpool.tile([128, 512], dt.float32, tag="scf",
                                      name=f"lt_{i}_{which}_{c}")
                    nc.vector.scalar_tensor_tensor(t1[:], rin[:, c, :], 0.0, mb[:],
                                                   AO.add, AO.subtract)
                    nc.vector.tensor_mul(q[:, c, :], t1[:], rsb[:])
                    if after_chunk is not None:
                        after_chunk(c)

            def body(first):
                # one shared kb: layer i+1's chunk-p write WAR-syncs against
                # layer i's chunk-p scores via subtile deps
                kb1 = spool.tile([128, CH, NK], dt.bfloat16, tag="kb", name="kb1")
                for i in range(L):
                    kbs[i] = kb1
                load_wkv(0)
                load_wkv(1)
                load_wq(0)
                if first:
                    load_late_consts()
                kv_v(0, range(16))
                for i in range(L):
                    layer(i)
                kbs.clear()
                wkv.clear()
                wq.clear()

            if nrep == 1:
                body(first=True)
            else:
                load_late_consts()
                with tc.For_i(0, nrep, 1):
                    for _u in range(unroll):
                        body(first=False)

    nc.compile()
    return nc


def _prep_core(inputs, core, host):
    b, qh = core // 2, core % 2
    sl = slice(qh * NT, (qh + 1) * NT)
    im = {
        "qT": _pack_fm(inputs["query"][sl, b, :].T).astype(BF),
        "vT8": host["v8"][b],
        "cq": _pack_fm(inputs["query_pos"][b, sl, :, 0].T).astype(BF),
        "sq": _pack_fm(host["sgn"] * inputs["query_pos"][b, sl, :, 1].T).astype(BF),
        "ck": _pack_fm(inputs["value_pos"][b, :, :, 0].T / W8SCALE).astype(BF),
        "sk": _pack_fm(inputs["value_pos"][b, :, :, 1].T / W8SCALE).astype(BF),
        "w8": host["w8"],
        "wts": host["wts"],
        "ada": host["ada"][b],
        "ind8": host["ind8"],
        "ones1": np.ones((1, 128), np.float32),
    }
    return im


def _prep_host(inputs):
    import concourse.mybir as mybir
    F8 = mybir.dt.np(mybir.dt.float8e4)

    wts = np.zeros((L, 4, 128, CH, C), BF)
    w8 = np.zeros((L, 3, 128, 2, 2, C), F8)
    for i in range(L):
        in_w, in_b = np.asarray(inputs["in_w"][i]), np.asarray(inputs["in_b"][i])
        wq = in_w[:C] * SCALING
        wk, wv = in_w[C:2 * C], in_w[2 * C:]
        if np.any(in_b):
            raise NotImplementedError("nonzero in-projection bias not supported")
        w8[i, 0] = _pack_w8(np.ascontiguousarray(wk.T) * W8SCALE).astype(F8)
        w8[i, 1] = _pack_w8(np.ascontiguousarray(_rot2_rows(wk).T) * W8SCALE).astype(F8)
        w8[i, 2] = _pack_w8(np.ascontiguousarray(wv.T) * W8SCALE).astype(F8)
        mats = [wq.T, np.asarray(inputs["out_w"][i]).T,
                np.asarray(inputs["w1"][i]).T, np.asarray(inputs["w2"][i]).T]
        for j, m in enumerate(mats):
            wts[i, j] = _pack_w(np.ascontiguousarray(m)).astype(BF)

    v8 = np.zeros((4, 128, 2, 2, NK), F8)
    for b in range(4):
        v8[b] = _pack_v8(np.asarray(inputs["value"][:, b, :], np.float32)).astype(F8)

    ada = np.zeros((4, 128, L * 4 * CH), np.float32)
    diff = np.asarray(inputs["diff_ts"], np.float32)
    for b in range(4):
        st = _silu(diff[b])
        for i in range(L):
            for qy, (aw, ab) in enumerate(
                    [(inputs["aw_attn"][i], inputs["ab_attn"][i]),
                     (inputs["aw_ffn"][i], inputs["ab_ffn"][i])]):
                mod = st @ np.asarray(aw, np.float32).T + np.asarray(ab, np.float32)
                sc, sh = 1.0 + mod[:C], mod[C:]
                for c in range(CH):
                    ada[b, :, (i * 4 + 2 * qy) * CH + c] = sc[c * 128:(c + 1) * 128]
                    ada[b, :, (i * 4 + 2 * qy + 1) * CH + c] = sh[c * 128:(c + 1) * 128]

    ind8 = np.zeros((8, C), np.float32)
    for h in range(H):
        base = (h // 2) * 128 + (h % 2) * 64
        ind8[h, base:base + 64] = 1.0
    sgn = np.ones((C, 1), np.float32)
    sgn[0::2] = -1.0

    flags = (bool(np.any(np.asarray(inputs["in_b"]))),
             bool(np.any(np.asarray(inputs["out_b"]))),
             bool(np.any(np.asarray(inputs["b1"]))),
             bool(np.any(np.asarray(inputs["b2"]))),
             bool(np.any(np.asarray(inputs["ln1_g"]) != 1.0) or np.any(np.asarray(inputs["ln2_g"]) != 1.0)),
             bool(np.any(np.asarray(inputs["ln1_b"])) or np.any(np.asarray(inputs["ln2_b"]))))
    return dict(wts=wts, w8=w8, v8=v8, ada=ada, ind8=ind8, sgn=sgn), flags


def _get_program(flags, nrep=1, unroll=1):
    key = (flags, nrep, unroll)
    if key not in _CACHE:
        _CACHE[key] = _build(flags, nrep, unroll)
    return _CACHE[key]


def _assemble(results):
    full = np.zeros((L, 1024, 4, C), np.float32)
    for core in range(8):
        b, qh = core // 2, core % 2
        arr = np.asarray(results[core]["out"], np.float32)   # [L, 128, CH, NT]
        fm = np.transpose(arr, (0, 2, 1, 3)).reshape(L, C, NT)
        full[:, qh * NT:(qh + 1) * NT, b, :] = np.transpose(fm, (0, 2, 1))
    return full


def kernel(**inputs):
    from concourse.bass_utils import run_bass_kernel_spmd

    inputs = {k: np.asarray(v) for k, v in inputs.items()}
    host, flags = _prep_host(inputs)
    nc = _get_program(flags)
    in_maps = [_prep_core(inputs, core, host) for core in range(8)]
    res = run_bass_kernel_spmd(nc, in_maps, list(range(8)))
    return _assemble(res.results)


# revision 82
# speedup vs baseline: 1.2006x; 1.2006x over previous
"""Trainium2 Bass kernel for nn_FFWRelativeCrossAttentionModule.

Sharding: 8 cores = (batch b in 0..3) x (query half qh in 0..1);
communication-free (attention only mixes query<->kv tokens).

v2 design notes (vs the v1 baseline):
- The K/V projection + K-rotary for layer i+1 is layer-independent, so it
  is emitted INSIDE layer i's attention pair loop to fill the PE idle
  gaps while ACT grinds through the softmax Exp (keeps the HAM clock
  warm; Exp on ACT is the hard floor of this kernel).
- K/V projections run in fp8e4m3 with MatmulPerfMode.DoubleRow
  (256-feature contraction per matmul): weights are host-packed
  [128,2sub,2kc,C] at x16 scale; the x1/16 is folded into the host-side
  ck/sk rotary codes (K) and the PSUM eviction scale (V).
- Elementwise work is spread across engines: ACT takes adaLN, ReLU and
  the LN copy/square (all in the pinned natural_log_exp_and_others
  table set); Pool (gpsimd) takes PSUM evictions / second half of the
  rotary; DVE keeps the shuffles and the fp32 residual path.
- 1/denominator is applied after out_proj (per-token scalar commutes
  through the feature contraction), removing the recip chain from the
  attention->oproj latency path.
- ck/sk/cq/sq stay resident in SBUF; weights are double-buffered so
  layer i+1's DMA overlaps layer i compute.
"""

import os
import sys

for _p in ("/opt/trn_rl_repo", "/root/.axon_site/_ro/trn_rl_repo"):
    if os.path.isdir(_p) and _p not in sys.path:
        sys.path.append(_p)

import numpy as np
import ml_dtypes

L, C, H, HD = 4, 512, 8, 64
NT = 512          # query tokens per core
NK = 2048         # kv tokens
CH = C // 128     # 4 chunks of 128 channels
SCALING = HD ** -0.5
W8SCALE = 16.0    # fp8 weight prescale
BF = ml_dtypes.bfloat16

_CACHE = {}


def _rot2_rows(w):
    # rot2(x)[2i] = -x[2i+1]; rot2(x)[2i+1] = x[2i], applied to the
    # projection output channels = rows of w.
    w2 = np.empty_like(w)
    w2[0::2] = -w[1::2]
    w2[1::2] = w[0::2]
    return w2


def _pack_w(m):
    # [C_in, C_out] -> [128, CH(kc), C_out] lhsT tile layout (bf16 weights)
    return np.ascontiguousarray(np.transpose(m.reshape(CH, 128, C), (1, 0, 2)))


def _pack_w8(m):
    # [C_in, C_out] -> [128, 2sub, 2kc, C_out] DoubleRow lhsT layout:
    # contraction feature f = kc*256 + sub*128 + p
    return np.ascontiguousarray(
        np.transpose(m.reshape(2, 2, 128, C), (2, 1, 0, 3)))


def _pack_fm(m):
    # feature-major [C, N] -> [128, CH, N]
    n = m.shape[1]
    return np.ascontiguousarray(np.transpose(m.reshape(CH, 128, n), (1, 0, 2)))


def _pack_v8(v):
    # token-major value [Nkv, C] -> [128, 2sub, 2kc, Nkv] with
    # vT8[p, s, c, t] = v[t, c*256 + s*128 + p]
    return np.ascontiguousarray(np.transpose(v.reshape(NK, 2, 2, 128), (3, 2, 1, 0)))


def _silu(x):
    return x / (1.0 + np.exp(-x))


def _pin_act_tables():
    """Make every activation resolve to natural_log_exp_and_others so the
    kernel uses one ACT table set (no ~1.3us reloads between funcs)."""
    from concourse import bacc as _bacc
    from concourse.hw_specs import get_activation_tables as _orig

    def patched(arch):
        tabs = _orig(arch)
        keep = "natural_log_exp_and_others"
        if keep in tabs:
            tabs = {k: (v if k == keep else set()) for k, v in tabs.items()}
        return tabs

    _bacc.get_activation_tables = patched


def _build(flags, nrep=1, unroll=1):
    import concourse.bass as bass
    import concourse.mybir as mybir
    import concourse.tile as tile
    from concourse import bacc

    _pin_act_tables()

    dt = mybir.dt
    AF = mybir.ActivationFunctionType
    AO = mybir.AluOpType
    DR = mybir.MatmulPerfMode.DoubleRow

    if any(flags):
        raise NotImplementedError("bias/ln-affine variants not supported")

    nc = bacc.Bacc("TRN2", target_bir_lowering=False, debug=False, num_devices=8)

    d_qT = nc.declare_dram_parameter("qT", [128, CH, NT], dt.bfloat16, isOutput=False)
    d_vT8 = nc.declare_dram_parameter("vT8", [128, 2, 2, NK], dt.float8e4, isOutput=False)
    d_cq = nc.declare_dram_parameter("cq", [128, CH, NT], dt.bfloat16, isOutput=False)
    d_sq = nc.declare_dram_parameter("sq", [128, CH, NT], dt.bfloat16, isOutput=False)
    d_ck = nc.declare_dram_parameter("ck", [128, CH, NK], dt.bfloat16, isOutput=False)
    d_sk = nc.declare_dram_parameter("sk", [128, CH, NK], dt.bfloat16, isOutput=False)
    d_w8 = nc.declare_dram_parameter("w8", [L, 3, 128, 2, 2, C], dt.float8e4, isOutput=False)
    d_wts = nc.declare_dram_parameter("wts", [L, 4, 128, CH, C], dt.bfloat16, isOutput=False)
    d_ada = nc.declare_dram_parameter("ada", [128, L * 4 * CH], dt.float32, isOutput=False)
    d_ind8 = nc.declare_dram_parameter("ind8", [8, C], dt.float32r, isOutput=False)
    d_ones1 = nc.declare_dram_parameter("ones1", [1, 128], dt.float32r, isOutput=False)
    d_out = nc.declare_dram_parameter("out", [L, 128, CH, NT], dt.bfloat16, isOutput=True)

    SWAP_MASK = [j + 1 if j % 2 == 0 else j - 1 for j in range(32)]
    WQ_NAMES = ["wq", "wo", "w1", "w2"]

    def ada_col(i, qty, c):
        return (i * 4 + qty) * CH + c

    with tile.TileContext(nc) as tc:
        with tc.tile_pool(name="const", bufs=1) as cpool, \
             tc.tile_pool(name="wkv", bufs=2) as kvpool, \
             tc.tile_pool(name="wq", bufs=2) as wqpool, \
             tc.tile_pool(name="state", bufs=1) as spool, \
             tc.tile_pool(name="epool", bufs=6) as epool, \
             tc.tile_pool(name="act", bufs=3) as apool, \
             tc.tile_pool(name="resid", bufs=2) as rpool, \
             tc.tile_pool(name="scr", bufs=4) as scrpool, \
             tc.tile_pool(name="scrf", bufs=2) as scfpool, \
             tc.tile_pool(name="small", bufs=2) as smpool, \
             tc.tile_pool(name="stats", bufs=3) as stpool, \
             tc.tile_pool(name="psum", bufs=2, space="PSUM") as ppool:

            # ---- resident constants; only the tensors the first PE work
            # needs are DMA'd here, the rest are issued inside body() so the
            # DMA queues aren't head-of-line blocked at kernel start ----
            ada = cpool.tile([128, L * 4 * CH], dt.float32)
            nc.sync.dma_start(ada[:], d_ada[:])
            vT8 = cpool.tile([128, 2, 2, NK], dt.float8e4)
            nc.sync.dma_start(vT8[:], d_vT8[:])
            cq = cpool.tile([128, CH, NT], dt.bfloat16)
            sq = cpool.tile([128, CH, NT], dt.bfloat16)
            ck = cpool.tile([128, CH, NK], dt.bfloat16)
            sk = cpool.tile([128, CH, NK], dt.bfloat16)
            indA = cpool.tile([6, C], dt.float32r)
            indB = cpool.tile([2, C], dt.float32r)
            ones_sc = cpool.tile([128, 1], dt.bfloat16)   # 1/C for mean matmuls
            nc.vector.memset(ones_sc[:], 1.0 / C)
            ones1 = cpool.tile([1, 128], dt.float32r)      # broadcast lhsT
            nc.sync.dma_start(ones1[:], d_ones1[:])

            # ---- persistent state ----
            q = spool.tile([128, CH, NT], dt.bfloat16)    # master q (feature-major)
            nc.sync.dma_start(q[:], d_qT[:])
            vb = spool.tile([128, NK // 128, H, HD + 1], dt.bfloat16)
            nc.vector.memset(vb[:, :, :, HD:HD + 1], 1.0)

            def load_late_consts():
                nc.sync.dma_start(cq[:], d_cq[:])
                nc.sync.dma_start(sq[:], d_sq[:])
                for _c in range(CH):
                    eng = nc.gpsimd if _c % 2 == 0 else nc.sync
                    eng.dma_start(ck[:, _c, :], d_ck[:, _c, :])
                    eng.dma_start(sk[:, _c, :], d_sk[:, _c, :])
                nc.sync.dma_start(indA[:], d_ind8[0:6, :])
                nc.sync.dma_start(indB[:], d_ind8[6:8, :])

            wkv = {}   # (layer, name) -> fp8 DoubleRow weight tile
            wq = {}    # (layer, name) -> bf16 weight tile
            kbs = {}   # layer -> rotary-embedded K.T tile

            def load_wkv(i):
                for j, nm in enumerate(["wk", "wk2", "wv"]):
                    t = kvpool.tile([128, 2, 2, C], dt.float8e4, tag=nm,
                                    name=f"{nm}8_{i}")
                    nc.sync.dma_start(t[:], d_w8[i, j])
                    wkv[(i, nm)] = t

            def load_wq(i):
                for j, nm in enumerate(WQ_NAMES):
                    t = wqpool.tile([128, CH, C], dt.bfloat16, tag=nm,
                                    name=f"{nm}_{i}")
                    nc.sync.dma_start(t[:], d_wts[i, j])
                    wq[(i, nm)] = t

            def kv_k_chunk(i, mc):
                # kproj chunk mc (fp8 DoubleRow, rot2 companion as a second
                # matmul) + rotary -> kbs[i][:, mc, :]
                kb = kbs[i]
                wk = wkv[(i, "wk")]
                wk2 = wkv[(i, "wk2")]
                for n in range(CH):
                    ksl = slice(n * 512, (n + 1) * 512)
                    pk = ppool.tile([128, 512], dt.float32, tag="kvmm",
                                    name=f"pk_{i}_{mc}_{n}")
                    for c in range(2):
                        nc.tensor.matmul(pk[:], wk[:, :, c, mc * 128:(mc + 1) * 128],
                                         vT8[:, :, c, ksl], start=(c == 0),
                                         stop=(c == 1), perf_mode=DR)
                    pk2 = ppool.tile([128, 512], dt.float32, tag="kvmm",
                                     name=f"pk2_{i}_{mc}_{n}")
                    for c in range(2):
                        nc.tensor.matmul(pk2[:], wk2[:, :, c, mc * 128:(mc + 1) * 128],
                                         vT8[:, :, c, ksl], start=(c == 0),
                                         stop=(c == 1), perf_mode=DR)
                    t1 = scrpool.tile([128, 512], dt.bfloat16, tag="scr",
                                      name=f"kc_{i}_{mc}_{n}")
                    nc.vector.scalar_tensor_tensor(
                        t1[:], pk[:], 0.0, ck[:, mc, ksl], AO.add, AO.mult)
                    t2 = scrpool.tile([128, 512], dt.bfloat16, tag="scr",
                                      name=f"ks_{i}_{mc}_{n}")
                    nc.vector.scalar_tensor_tensor(
                        t2[:], pk2[:], 0.0, sk[:, mc, ksl], AO.add, AO.mult)
                    nc.gpsimd.tensor_tensor(kb[:, mc, ksl], t1[:], t2[:], AO.add)

            def kv_v(i, idxs):
                # vproj (fp8 DoubleRow, token-major) -> vb ; x1/16 on evict.
                # Evictions stay off ACT so they never queue-block the LN
                # Ln/Exp chain.
                wv = wkv[(i, "wv")]
                for idx in idxs:
                    pv = ppool.tile([128, 512], dt.float32, tag="kvmm",
                                    name=f"pv_{i}_{idx}")
                    for c in range(2):
                        nc.tensor.matmul(pv[:], vT8[:, :, c, idx * 128:(idx + 1) * 128],
                                         wv[:, :, c, :], start=(c == 0),
                                         stop=(c == 1), perf_mode=DR)
                    if idx % 2 == 0:
                        nc.vector.tensor_scalar_mul(
                            vb[:, idx, :, 0:HD],
                            pv[:].rearrange("p (h d) -> p h d", h=H), 1.0 / W8SCALE)
                    else:
                        nc.scalar.mul(
                            vb[:, idx, :, 0:HD],
                            pv[:].rearrange("p (h d) -> p h d", h=H), 1.0 / W8SCALE)

            def layer(i):
                if i + 1 < L:
                    load_wq(i + 1)
                if i + 2 < L:
                    load_wkv(i + 2)

                # ---- adaln (attn) on DVE (4x mode: all-bf16 SBUF) ----
                xa = apool.tile([128, CH, NT], dt.bfloat16, tag="act", name=f"xa_{i}")
                for c in range(CH):
                    nc.vector.tensor_scalar(
                        xa[:, c, :], q[:, c, :],
                        ada[:, ada_col(i, 0, c):ada_col(i, 0, c) + 1],
                        ada[:, ada_col(i, 1, c):ada_col(i, 1, c) + 1],
                        AO.mult, AO.add)

                # ---- q projection + rotary -> qb ----
                qb = spool.tile([128, CH, NT], dt.bfloat16, tag="qb", name=f"qb_{i}")
                for mc in range(CH):
                    pq = ppool.tile([128, 512], dt.float32, tag="kvmm",
                                    name=f"pq_{i}_{mc}")
                    for kc in range(CH):
                        nc.tensor.matmul(pq[:], wq[(i, "wq")][:, kc, mc * 128:(mc + 1) * 128],
                                         xa[:, kc, :], start=(kc == 0), stop=(kc == CH - 1))
                    t1 = scrpool.tile([128, 512], dt.bfloat16, tag="scr",
                                      name=f"qc_{i}_{mc}")
                    nc.vector.scalar_tensor_tensor(
                        t1[:], pq[:], 0.0, cq[:, mc, :], AO.add, AO.mult)
                    qsh = scfpool.tile([128, 512], dt.float32, tag="ksh",
                                       name=f"qsh_{i}_{mc}")
                    nc.vector.stream_shuffle(qsh[:], pq[:], SWAP_MASK)
                    t2 = scrpool.tile([128, 512], dt.bfloat16, tag="scr",
                                      name=f"qs_{i}_{mc}")
                    nc.vector.scalar_tensor_tensor(
                        t2[:], qsh[:], 0.0, sq[:, mc, :], AO.add, AO.mult)
                    nc.gpsimd.tensor_tensor(qb[:, mc, :], t1[:], t2[:], AO.add)

                # ---- attention (+ next layer's K-proj interleaved) ----
                kb = kbs[i]
                o_raw = apool.tile([128, CH, NT], dt.bfloat16, tag="act",
                                   name=f"oraw_{i}")
                # pairs 0-2 and pair 3 get separate base-0 den/recip tiles:
                # group A's 1/den computes mid-attention, group B right at the
                # end — both ready before their rb matmuls in the post phase
                den2t = [smpool.tile([6, 512], dt.float32r, tag="den0", bufs=1,
                                     name=f"den0_{i}"),
                         smpool.tile([2, 512], dt.float32r, tag="den1", bufs=1,
                                     name=f"den1_{i}")]
                for p in range(H // 2):
                    hpair = (2 * p, 2 * p + 1)
                    if i == 0:
                        kv_k_chunk(0, p)
                    oacc = {h: ppool.tile([128, 512], dt.float32, tag="mm",
                                          name=f"oacc_{i}_{h}") for h in hpair}
                    Eg = {}

                    def attn_v(g):
                        for h in hpair:
                            for j in range(2):
                                nidx = g * 2 + j
                                nc.tensor.matmul(
                                    oacc[h][0:HD + 1, :], vb[:, nidx, h, :],
                                    Eg[(h, g)][:, j, :], start=(nidx == 0),
                                    stop=(nidx == 15), skip_group_check=True)

                    for g in range(8):
                        ps = {h: ppool.tile([128, 2, 512], dt.float32, tag="sc",
                                            name=f"ps_{i}_{h}_{g}") for h in hpair}
                        for j in range(2):
                            nidx = g * 2 + j
                            for h in hpair:
                                off = (h % 2) * 64
                                nc.tensor.matmul(
                                    ps[h][:, j, :],
                                    kb[off:off + 64, p, nidx * 128:(nidx + 1) * 128],
                                    qb[off:off + 64, p, :], start=True, stop=True)
                        for h in hpair:
                            E = epool.tile([128, 2, 512], dt.bfloat16, tag="E",
                                           name=f"E_{i}_{h}_{g}")
                            nc.scalar.activation(E[:], ps[h][:], AF.Exp)
                            Eg[(h, g)] = E
                        if g >= 2:
                            attn_v(g - 2)
                    attn_v(6)
                    attn_v(7)
                    half = 0 if p < 3 else 1
                    for h in hpair:
                        off = (h % 2) * 64
                        nc.vector.tensor_copy(o_raw[off:off + 64, p, :], oacc[h][0:HD, :])
                        dh = smpool.tile([1, 512], dt.float32r, tag="denh", bufs=1,
                                         name=f"dh_{i}_{h}")
                        nc.vector.tensor_copy(dh[:], oacc[h][HD:HD + 1, :])
                        row = (2 * p + h % 2) if p < 3 else (h % 2)
                        nc.sync.dma_start(den2t[half][row:row + 1, :], dh[:])
                    if p == 2 or p == 3:
                        # 1/den for this group, in place, inside the exp
                        # window (off the post critical path)
                        nc.scalar.activation(den2t[half][:], den2t[half][:], AF.Ln)
                        nc.scalar.activation(den2t[half][:], den2t[half][:],
                                             AF.Exp, scale=-1.0)
                    if i + 1 < L:
                        kv_k_chunk(i + 1, p)

                # ---- out projection; r1 = po*recip + q ; fused LN1 stats.
                # rb/finish lag one chunk behind po so the PE queue never
                # head-of-line blocks on the recip chain or evictions.
                r1 = rpool.tile([128, CH, NT], dt.float32, tag="resid", name=f"r1_{i}")
                pm1 = ppool.tile([128, 512], dt.float32, tag="mm", name=f"pm1_{i}")
                pv1 = ppool.tile([128, 512], dt.float32, tag="mm", name=f"pv1_{i}")
                store1 = {}
                po = {}

                def fin1(mo):
                    ind = indA if mo < 3 else indB
                    rb = ppool.tile([128, 512], dt.float32, tag="sc", name=f"rb_{i}_{mo}")
                    nc.tensor.matmul(rb[:], ind[:, mo * 128:(mo + 1) * 128],
                                     den2t[0 if mo < 3 else 1][:], start=True, stop=True)
                    rb_s = scrpool.tile([128, 512], dt.float32, tag="rbs", bufs=2,
                                        name=f"rbs_{i}_{mo}")
                    nc.scalar.copy(rb_s[:], rb[:])
                    t = scfpool.tile([128, 512], dt.float32, tag="ksh",
                                     name=f"rt_{i}_{mo}")
                    nc.vector.tensor_mul(t[:], po[mo][:], rb_s[:])
                    nc.gpsimd.tensor_tensor(r1[:, mo, :], t[:], q[:, mo, :], AO.add)
                    _ln_chunk_evict(i, 0, r1, mo, store1)
                    if mo >= 1:
                        _ln_stats_mm(store1, mo - 1, pm1, pv1)

                for mo in range(CH):
                    po[mo] = ppool.tile([128, 512], dt.float32, tag="kvmm",
                                        name=f"po_{i}_{mo}")
                    for kc in range(CH):
                        nc.tensor.matmul(po[mo][:], wq[(i, "wo")][:, kc, mo * 128:(mo + 1) * 128],
                                         o_raw[:, kc, :], start=(kc == 0), stop=(kc == CH - 1))
                    if mo >= 1:
                        fin1(mo - 1)
                fin1(CH - 1)
                _ln_stats_mm(store1, CH - 1, pm1, pv1)
                ln1 = _ln_head(i, 0, pm1, pv1)

                # ---- next layer's V projection, first half (PE slack while
                # the LN1 stats chain resolves) ----
                if i + 1 < L:
                    kv_v(i + 1, range(0, 8))

                # ---- LN1 -> q, with per-chunk adaln(ffn) follow-on ----
                xfb = apool.tile([128, CH, NT], dt.bfloat16, tag="act", name=f"xfb_{i}")

                def emit_xfb(c):
                    nc.vector.tensor_scalar(
                        xfb[:, c, :], q[:, c, :],
                        ada[:, ada_col(i, 2, c):ada_col(i, 2, c) + 1],
                        ada[:, ada_col(i, 3, c):ada_col(i, 3, c) + 1],
                        AO.mult, AO.add)

                _ln_apply(i, 0, r1, *ln1, after_chunk=emit_xfb)

                # ---- FFN ; fused LN2 stats ----
                hbf = apool.tile([128, CH, NT], dt.bfloat16, tag="act", name=f"hbf_{i}")
                for mh in range(CH):
                    ph = ppool.tile([128, 512], dt.float32, tag="kvmm", name=f"ph_{i}_{mh}")
                    for kc in range(CH):
                        nc.tensor.matmul(ph[:], wq[(i, "w1")][:, kc, mh * 128:(mh + 1) * 128],
                                         xfb[:, kc, :], start=(kc == 0), stop=(kc == CH - 1))
                    nc.vector.tensor_scalar_max(hbf[:, mh, :], ph[:], 0.0)
                r2 = rpool.tile([128, CH, NT], dt.float32, tag="resid", name=f"r2_{i}")
                pm2 = ppool.tile([128, 512], dt.float32, tag="mm", name=f"pm2_{i}")
                pv2 = ppool.tile([128, 512], dt.float32, tag="mm", name=f"pv2_{i}")
                store2 = {}
                pf = {}

                def fin2(mo):
                    nc.vector.tensor_add(r2[:, mo, :], pf[mo][:], xfb[:, mo, :])
                    _ln_chunk_evict(i, 1, r2, mo, store2)
                    if mo >= 1:
                        _ln_stats_mm(store2, mo - 1, pm2, pv2)

                for mo in range(CH):
                    pf[mo] = ppool.tile([128, 512], dt.float32, tag="kvmm",
                                        name=f"pf_{i}_{mo}")
                    for kc in range(CH):
                        nc.tensor.matmul(pf[mo][:], wq[(i, "w2")][:, kc, mo * 128:(mo + 1) * 128],
                                         hbf[:, kc, :], start=(kc == 0), stop=(kc == CH - 1))
                    if mo >= 1:
                        fin2(mo - 1)
                fin2(CH - 1)
                _ln_stats_mm(store2, CH - 1, pm2, pv2)
                ln2 = _ln_head(i, 1, pm2, pv2)

                # ---- next layer's V projection, second half (PE slack while
                # the LN2 stats chain resolves) ----
                if i + 1 < L:
                    kv_v(i + 1, range(8, 16))

                # ---- LN2 -> q ; emit layer output per chunk ----
                _ln_apply(i, 1, r2, *ln2,
                          after_chunk=lambda c: nc.sync.dma_start(d_out[i, :, c, :], q[:, c, :]))

            def _ln_chunk_evict(i, which, rin, c, store):
                # bf16 copy + square right after rin chunk lands (DVE + Pool)
                rbf = scrpool.tile([128, 512], dt.bfloat16, tag="scr",
                                   name=f"rbf_{i}_{which}_{c}")
                nc.vector.tensor_copy(rbf[:], rin[:, c, :])
                r2b = scrpool.tile([128, 512], dt.bfloat16, tag="scr",
                                   name=f"r2b_{i}_{which}_{c}")
                nc.vector.tensor_mul(r2b[:], rbf[:], rbf[:])
                store[c] = (rbf, r2b)

            def _ln_stats_mm(store, c, pm, pv):
                # lag-1 accumulation so the PE queue rarely waits on evicts
                rbf, r2b = store[c]
                nc.tensor.matmul(pm[0:1, :], ones_sc[:], rbf[:],
                                 start=(c == 0), stop=(c == CH - 1),
                                 skip_group_check=True)
                nc.tensor.matmul(pv[0:1, :], ones_sc[:], r2b[:],
                                 start=(c == 0), stop=(c == CH - 1),
                                 skip_group_check=True)

            def _ln_head(i, which, pm, pv):
                # stat chain emitted FIRST so the DVE/ACT queue heads aren't
                # blocked behind filler work
                m_sb = stpool.tile([1, 512], dt.float32r, tag="st", name=f"m_{i}_{which}")
                nc.vector.tensor_copy(m_sb[:], pm[0:1, :])
                msq = stpool.tile([1, 512], dt.float32, tag="st", name=f"msq_{i}_{which}")
                nc.scalar.activation(msq[:], pm[0:1, :], AF.Square)
                var = stpool.tile([1, 512], dt.float32, tag="st", name=f"var_{i}_{which}")
                nc.vector.scalar_tensor_tensor(var[:], pv[0:1, :], 1e-5, msq[:],
                                               AO.add, AO.subtract)
                lnv = stpool.tile([1, 512], dt.float32, tag="st", name=f"lnv_{i}_{which}")
                nc.scalar.activation(lnv[:], var[:], AF.Ln)
                rstd = stpool.tile([1, 512], dt.float32r, tag="st", name=f"rstd_{i}_{which}")
                nc.scalar.activation(rstd[:], lnv[:], AF.Exp, scale=-0.5)
                return m_sb, rstd

            def _ln_apply(i, which, rin, m_sb, rstd, after_chunk=None):
                mb = ppool.tile([128, 512], dt.float32, tag="sc", name=f"mb_{i}_{which}")
                nc.tensor.matmul(mb[:], ones1[:], m_sb[:], start=True, stop=True)
                rsb = ppool.tile([128, 512], dt.float32, tag="sc", name=f"rsb_{i}_{which}")
                nc.tensor.matmul(rsb[:], ones1[:], rstd[:], start=True, stop=True)
                for c in range(CH):
                    t1 = scfpool.tile([128, 512], dt.float32, tag="scf",
                                      name=f"lt_{i}_{which}_{c}")
                    nc.vector.scalar_tensor_tensor(t1[:], rin[:, c, :], 0.0, mb[:],
                                                   AO.add, AO.subtract)
                    nc.vector.tensor_mul(q[:, c, :], t1[:], rsb[:])
                    if after_chunk is not None:
                        after_chunk(c)

            def body(first):
                # one shared kb: layer i+1's chunk-p write WAR-syncs against
                # layer i's chunk-p scores via subtile deps
                kb1 = spool.tile([128, CH, NK], dt.bfloat16, tag="kb", name="kb1")
                for i in range(L):
                    kbs[i] = kb1
                load_wkv(0)
                load_wkv(1)
                load_wq(0)
                if first:
                    load_late_consts()
                kv_v(0, range(16))
                for i in range(L):
                    layer(i)
                kbs.clear()
                wkv.clear()
                wq.clear()

            if nrep == 1:
                body(first=True)
            else:
                load_late_consts()
                with tc.For_i(0, nrep, 1):
                    for _u in range(unroll):
                        body(first=False)

    nc.compile()
    return nc


def _prep_core(inputs, core, host):
    b, qh = core // 2, core % 2
    sl = slice(qh * NT, (qh + 1) * NT)
    im = {
        "qT": _pack_fm(inputs["query"][sl, b, :].T).astype(BF),
        "vT8": host["v8"][b],
        "cq": _pack_fm(inputs["query_pos"][b, sl, :, 0].T).astype(BF),
        "sq": _pack_fm(host["sgn"] * inputs["query_pos"][b, sl, :, 1].T).astype(BF),
        "ck": _pack_fm(inputs["value_pos"][b, :, :, 0].T / W8SCALE).astype(BF),
        "sk": _pack_fm(inputs["value_pos"][b, :, :, 1].T / W8SCALE).astype(BF),
        "w8": host["w8"],
        "wts": host["wts"],
        "ada": host["ada"][b],
        "ind8": host["ind8"],
        "ones1": np.ones((1, 128), np.float32),
    }
    return im


def _prep_host(inputs):
    import concourse.mybir as mybir
    F8 = mybir.dt.np(mybir.dt.float8e4)

    wts = np.zeros((L, 4, 128, CH, C), BF)
    w8 = np.zeros((L, 3, 128, 2, 2, C), F8)
    for i in range(L):
        in_w, in_b = np.asarray(inputs["in_w"][i]), np.asarray(inputs["in_b"][i])
        wq = in_w[:C] * SCALING
        wk, wv = in_w[C:2 * C], in_w[2 * C:]
        if np.any(in_b):
            raise NotImplementedError("nonzero in-projection bias not supported")
        w8[i, 0] = _pack_w8(np.ascontiguousarray(wk.T) * W8SCALE).astype(F8)
        w8[i, 1] = _pack_w8(np.ascontiguousarray(_rot2_rows(wk).T) * W8SCALE).astype(F8)
        w8[i, 2] = _pack_w8(np.ascontiguousarray(wv.T) * W8SCALE).astype(F8)
        mats = [wq.T, np.asarray(inputs["out_w"][i]).T,
                np.asarray(inputs["w1"][i]).T, np.asarray(inputs["w2"][i]).T]
        for j, m in enumerate(mats):
            wts[i, j] = _pack_w(np.ascontiguousarray(m)).astype(BF)

    v8 = np.zeros((4, 128, 2, 2, NK), F8)
    for b in range(4):
        v8[b] = _pack_v8(np.asarray(inputs["value"][:, b, :], np.float32)).astype(F8)

    ada = np.zeros((4, 128, L * 4 * CH), np.float32)
    diff = np.asarray(inputs["diff_ts"], np.float32)
    for b in range(4):
        st = _silu(diff[b])
        for i in range(L):
            for qy, (aw, ab) in enumerate(
                    [(inputs["aw_attn"][i], inputs["ab_attn"][i]),
                     (inputs["aw_ffn"][i], inputs["ab_ffn"][i])]):
                mod = st @ np.asarray(aw, np.float32).T + np.asarray(ab, np.float32)
                sc, sh = 1.0 + mod[:C], mod[C:]
                for c in range(CH):
                    ada[b, :, (i * 4 + 2 * qy) * CH + c] = sc[c * 128:(c + 1) * 128]
                    ada[b, :, (i * 4 + 2 * qy + 1) * CH + c] = sh[c * 128:(c + 1) * 128]

    ind8 = np.zeros((8, C), np.float32)
    for h in range(H):
        base = (h // 2) * 128 + (h % 2) * 64
        ind8[h, base:base + 64] = 1.0
    sgn = np.ones((C, 1), np.float32)
    sgn[0::2] = -1.0

    flags = (bool(np.any(np.asarray(inputs["in_b"]))),
             bool(np.any(np.asarray(inputs["out_b"]))),
             bool(np.any(np.asarray(inputs["b1"]))),
             bool(np.any(np.asarray(inputs["b2"]))),
             bool(np.any(np.asarray(inputs["ln1_g"]) != 1.0) or np.any(np.asarray(inputs["ln2_g"]) != 1.0)),
             bool(np.any(np.asarray(inputs["ln1_b"])) or np.any(np.asarray(inputs["ln2_b"]))))
    return dict(wts=wts, w8=w8, v8=v8, ada=ada, ind8=ind8, sgn=sgn), flags


def _get_program(flags, nrep=1, unroll=1):
    key = (flags, nrep, unroll)
    if key not in _CACHE:
        _CACHE[key] = _build(flags, nrep, unroll)
    return _CACHE[key]


def _assemble(results):
    full = np.zeros((L, 1024, 4, C), np.float32)
    for core in range(8):
        b, qh = core // 2, core % 2
        arr = np.asarray(results[core]["out"], np.float32)   # [L, 128, CH, NT]
        fm = np.transpose(arr, (0, 2, 1, 3)).reshape(L, C, NT)
        full[:, qh * NT:(qh + 1) * NT, b, :] = np.transpose(fm, (0, 2, 1))
    return full


def kernel(**inputs):
    from concourse.bass_utils import run_bass_kernel_spmd

    inputs = {k: np.asarray(v) for k, v in inputs.items()}
    host, flags = _prep_host(inputs)
    nc = _get_program(flags)
    in_maps = [_prep_core(inputs, core, host) for core in range(8)]
    res = run_bass_kernel_spmd(nc, in_maps, list(range(8)))
    return _assemble(res.results)


# revision 87
# speedup vs baseline: 1.2690x; 1.0570x over previous
"""Trainium2 Bass kernel for nn_FFWRelativeCrossAttentionModule.

Sharding: 8 cores = (batch b in 0..3) x (query half qh in 0..1);
communication-free (attention only mixes query<->kv tokens).

v2 design notes (vs the v1 baseline):
- The K/V projection + K-rotary for layer i+1 is layer-independent, so it
  is emitted INSIDE layer i's attention pair loop to fill the PE idle
  gaps while ACT grinds through the softmax Exp (keeps the HAM clock
  warm; Exp on ACT is the hard floor of this kernel).
- K/V projections run in fp8e4m3 with MatmulPerfMode.DoubleRow
  (256-feature contraction per matmul): weights are host-packed
  [128,2sub,2kc,C] at x16 scale; the x1/16 is folded into the host-side
  ck/sk rotary codes (K) and the PSUM eviction scale (V).
- Elementwise work is spread across engines: ACT takes adaLN, ReLU and
  the LN copy/square (all in the pinned natural_log_exp_and_others
  table set); Pool (gpsimd) takes PSUM evictions / second half of the
  rotary; DVE keeps the shuffles and the fp32 residual path.
- 1/denominator is applied after out_proj (per-token scalar commutes
  through the feature contraction), removing the recip chain from the
  attention->oproj latency path.
- ck/sk/cq/sq stay resident in SBUF; weights are double-buffered so
  layer i+1's DMA overlaps layer i compute.
"""

import os
import sys

for _p in ("/opt/trn_rl_repo", "/root/.axon_site/_ro/trn_rl_repo"):
    if os.path.isdir(_p) and _p not in sys.path:
        sys.path.append(_p)

import numpy as np
import ml_dtypes

L, C, H, HD = 4, 512, 8, 64
NT = 512          # query tokens per core
NK = 2048         # kv tokens
CH = C // 128     # 4 chunks of 128 channels
SCALING = HD ** -0.5
W8SCALE = 16.0    # fp8 weight prescale
BF = ml_dtypes.bfloat16

_CACHE = {}


def _rot2_rows(w):
    # rot2(x)[2i] = -x[2i+1]; rot2(x)[2i+1] = x[2i], applied to the
    # projection output channels = rows of w.
    w2 = np.empty_like(w)
    w2[0::2] = -w[1::2]
    w2[1::2] = w[0::2]
    return w2


def _pack_w(m):
    # [C_in, C_out] -> [128, CH(kc), C_out] lhsT tile layout (bf16 weights)
    return np.ascontiguousarray(np.transpose(m.reshape(CH, 128, C), (1, 0, 2)))


def _pack_w8(m):
    # [C_in, C_out] -> [128, 2sub, 2kc, C_out] DoubleRow lhsT layout:
    # contraction feature f = kc*256 + sub*128 + p
    return np.ascontiguousarray(
        np.transpose(m.reshape(2, 2, 128, C), (2, 1, 0, 3)))


def _pack_fm(m):
    # feature-major [C, N] -> [128, CH, N]
    n = m.shape[1]
    return np.ascontiguousarray(np.transpose(m.reshape(CH, 128, n), (1, 0, 2)))


def _pack_v8(v):
    # token-major value [Nkv, C] -> [128, 2sub, 2kc, Nkv] with
    # vT8[p, s, c, t] = v[t, c*256 + s*128 + p]
    return np.ascontiguousarray(np.transpose(v.reshape(NK, 2, 2, 128), (3, 2, 1, 0)))


def _silu(x):
    return x / (1.0 + np.exp(-x))


def _pin_act_tables():
    """Make every activation resolve to natural_log_exp_and_others so the
    kernel uses one ACT table set (no ~1.3us reloads between funcs)."""
    from concourse import bacc as _bacc
    from concourse.hw_specs import get_activation_tables as _orig

    def patched(arch):
        tabs = _orig(arch)
        keep = "natural_log_exp_and_others"
        if keep in tabs:
            tabs = {k: (v if k == keep else set()) for k, v in tabs.items()}
        return tabs

    _bacc.get_activation_tables = patched


def _build(flags, nrep=1, unroll=1):
    import concourse.bass as bass
    import concourse.mybir as mybir
    import concourse.tile as tile
    from concourse import bacc

    _pin_act_tables()

    dt = mybir.dt
    AF = mybir.ActivationFunctionType
    AO = mybir.AluOpType
    DR = mybir.MatmulPerfMode.DoubleRow

    if any(flags):
        raise NotImplementedError("bias/ln-affine variants not supported")

    nc = bacc.Bacc("TRN2", target_bir_lowering=False, debug=False, num_devices=8)

    d_qT = nc.declare_dram_parameter("qT", [128, CH, NT], dt.bfloat16, isOutput=False)
    d_vT8 = nc.declare_dram_parameter("vT8", [128, 2, 2, NK], dt.float8e4, isOutput=False)
    d_cq = nc.declare_dram_parameter("cq", [128, CH, NT], dt.bfloat16, isOutput=False)
    d_sq = nc.declare_dram_parameter("sq", [128, CH, NT], dt.bfloat16, isOutput=False)
    d_ck = nc.declare_dram_parameter("ck", [128, CH, NK], dt.bfloat16, isOutput=False)
    d_sk = nc.declare_dram_parameter("sk", [128, CH, NK], dt.bfloat16, isOutput=False)
    d_w8 = nc.declare_dram_parameter("w8", [L, 3, 128, 2, 2, C], dt.float8e4, isOutput=False)
    d_wts = nc.declare_dram_parameter("wts", [L, 4, 128, CH, C], dt.bfloat16, isOutput=False)
    d_ada = nc.declare_dram_parameter("ada", [128, L * 4 * CH], dt.float32, isOutput=False)
    d_ind8 = nc.declare_dram_parameter("ind8", [8, C], dt.float32r, isOutput=False)
    d_ones1 = nc.declare_dram_parameter("ones1", [1, 128], dt.float32r, isOutput=False)
    d_out = nc.declare_dram_parameter("out", [L, 128, CH, NT], dt.bfloat16, isOutput=True)

    SWAP_MASK = [j + 1 if j % 2 == 0 else j - 1 for j in range(32)]
    WQ_NAMES = ["wq", "wo", "w1", "w2"]

    def ada_col(i, qty, c):
        return (i * 4 + qty) * CH + c

    with tile.TileContext(nc) as tc:
        with tc.tile_pool(name="const", bufs=1) as cpool, \
             tc.tile_pool(name="wkv", bufs=2) as kvpool, \
             tc.tile_pool(name="wq", bufs=2) as wqpool, \
             tc.tile_pool(name="state", bufs=1) as spool, \
             tc.tile_pool(name="epool", bufs=6) as epool, \
             tc.tile_pool(name="act", bufs=3) as apool, \
             tc.tile_pool(name="resid", bufs=2) as rpool, \
             tc.tile_pool(name="scr", bufs=4) as scrpool, \
             tc.tile_pool(name="scrf", bufs=2) as scfpool, \
             tc.tile_pool(name="small", bufs=2) as smpool, \
             tc.tile_pool(name="stats", bufs=3) as stpool, \
             tc.tile_pool(name="psum", bufs=2, space="PSUM") as ppool:

            # ---- resident constants; only the tensors the first PE work
            # needs are DMA'd here, the rest are issued inside body() so the
            # DMA queues aren't head-of-line blocked at kernel start ----
            ada = cpool.tile([128, L * 4 * CH], dt.float32)
            nc.sync.dma_start(ada[:], d_ada[:])
            vT8 = cpool.tile([128, 2, 2, NK], dt.float8e4)
            nc.sync.dma_start(vT8[:], d_vT8[:])
            cq = cpool.tile([128, CH, NT], dt.bfloat16)
            sq = cpool.tile([128, CH, NT], dt.bfloat16)
            ck = cpool.tile([128, CH, NK], dt.bfloat16)
            sk = cpool.tile([128, CH, NK], dt.bfloat16)
            indA = cpool.tile([6, C], dt.float32r)
            indB = cpool.tile([2, C], dt.float32r)
            ones_sc = cpool.tile([128, 1], dt.bfloat16)   # 1/C for mean matmuls
            nc.vector.memset(ones_sc[:], 1.0 / C)
            ones1 = cpool.tile([1, 128], dt.float32r)      # broadcast lhsT
            nc.sync.dma_start(ones1[:], d_ones1[:])

            # ---- persistent state ----
            q = spool.tile([128, CH, NT], dt.bfloat16)    # master q (feature-major)
            nc.sync.dma_start(q[:], d_qT[:])
            vb = spool.tile([128, NK // 128, H, HD + 1], dt.bfloat16)
            nc.vector.memset(vb[:, :, :, HD:HD + 1], 1.0)

            def load_late_consts():
                nc.sync.dma_start(cq[:], d_cq[:])
                nc.sync.dma_start(sq[:], d_sq[:])
                for _c in range(CH):
                    eng = nc.gpsimd if _c % 2 == 0 else nc.sync
                    eng.dma_start(ck[:, _c, :], d_ck[:, _c, :])
                    eng.dma_start(sk[:, _c, :], d_sk[:, _c, :])
                nc.sync.dma_start(indA[:], d_ind8[0:6, :])
                nc.sync.dma_start(indB[:], d_ind8[6:8, :])

            wkv = {}   # (layer, name) -> fp8 DoubleRow weight tile
            wq = {}    # (layer, name) -> bf16 weight tile
            kbs = {}   # layer -> rotary-embedded K.T tile

            def load_wkv(i):
                for j, nm in enumerate(["wk", "wk2", "wv"]):
                    t = kvpool.tile([128, 2, 2, C], dt.float8e4, tag=nm,
                                    name=f"{nm}8_{i}")
                    nc.sync.dma_start(t[:], d_w8[i, j])
                    wkv[(i, nm)] = t

            def load_wq(i):
                for j, nm in enumerate(WQ_NAMES):
                    t = wqpool.tile([128, CH, C], dt.bfloat16, tag=nm,
                                    name=f"{nm}_{i}")
                    nc.sync.dma_start(t[:], d_wts[i, j])
                    wq[(i, nm)] = t

            def kv_k_chunk(i, mc):
                # kproj chunk mc (fp8 DoubleRow, rot2 companion as a second
                # matmul) + rotary -> kbs[i][:, mc, :]
                kb = kbs[i]
                wk = wkv[(i, "wk")]
                wk2 = wkv[(i, "wk2")]
                for n in range(CH):
                    ksl = slice(n * 512, (n + 1) * 512)
                    pk = ppool.tile([128, 512], dt.float32, tag="kvmm",
                                    name=f"pk_{i}_{mc}_{n}")
                    for c in range(2):
                        nc.tensor.matmul(pk[:], wk[:, :, c, mc * 128:(mc + 1) * 128],
                                         vT8[:, :, c, ksl], start=(c == 0),
                                         stop=(c == 1), perf_mode=DR)
                    t1 = scrpool.tile([128, 512], dt.bfloat16, tag="scr",
                                      name=f"kc_{i}_{mc}_{n}")
                    nc.vector.scalar_tensor_tensor(
                        t1[:], pk[:], 0.0, ck[:, mc, ksl], AO.add, AO.mult)
                    pk2 = ppool.tile([128, 512], dt.float32, tag="kvmm",
                                     name=f"pk2_{i}_{mc}_{n}")
                    for c in range(2):
                        nc.tensor.matmul(pk2[:], wk2[:, :, c, mc * 128:(mc + 1) * 128],
                                         vT8[:, :, c, ksl], start=(c == 0),
                                         stop=(c == 1), perf_mode=DR)
                    t2 = scrpool.tile([128, 512], dt.bfloat16, tag="scr",
                                      name=f"ks_{i}_{mc}_{n}")
                    nc.vector.scalar_tensor_tensor(
                        t2[:], pk2[:], 0.0, sk[:, mc, ksl], AO.add, AO.mult)
                    nc.gpsimd.tensor_tensor(kb[:, mc, ksl], t1[:], t2[:], AO.add)

            def kv_v(i, idxs):
                # vproj (fp8 DoubleRow, token-major) -> vb ; x1/16 on evict.
                # Evictions stay off ACT so they never queue-block the LN
                # Ln/Exp chain.
                wv = wkv[(i, "wv")]
                for idx in idxs:
                    # alternate PSUM rings: the sc ring is idle during the LN
                    # windows, doubling the eviction pipeline depth
                    pv = ppool.tile([128, 512], dt.float32,
                                    tag="kvmm" if idx % 2 == 0 else "sc",
                                    name=f"pv_{i}_{idx}")
                    for c in range(2):
                        nc.tensor.matmul(pv[:], vT8[:, :, c, idx * 128:(idx + 1) * 128],
                                         wv[:, :, c, :], start=(c == 0),
                                         stop=(c == 1), perf_mode=DR)
                    if idx % 2 == 0:
                        nc.vector.tensor_scalar_mul(
                            vb[:, idx, :, 0:HD],
                            pv[:].rearrange("p (h d) -> p h d", h=H), 1.0 / W8SCALE)
                    else:
                        nc.scalar.mul(
                            vb[:, idx, :, 0:HD],
                            pv[:].rearrange("p (h d) -> p h d", h=H), 1.0 / W8SCALE)

            def layer(i):
                if i + 1 < L:
                    load_wq(i + 1)
                if i + 2 < L:
                    load_wkv(i + 2)

                # ---- adaln (attn) on DVE (4x mode: all-bf16 SBUF) ----
                xa = apool.tile([128, CH, NT], dt.bfloat16, tag="act", name=f"xa_{i}")
                for c in range(CH):
                    nc.vector.tensor_scalar(
                        xa[:, c, :], q[:, c, :],
                        ada[:, ada_col(i, 0, c):ada_col(i, 0, c) + 1],
                        ada[:, ada_col(i, 1, c):ada_col(i, 1, c) + 1],
                        AO.mult, AO.add)

                # ---- q projection + rotary -> qb ----
                qb = spool.tile([128, CH, NT], dt.bfloat16, tag="qb", name=f"qb_{i}")
                for mc in range(CH):
                    pq = ppool.tile([128, 512], dt.float32, tag="kvmm",
                                    name=f"pq_{i}_{mc}")
                    for kc in range(CH):
                        nc.tensor.matmul(pq[:], wq[(i, "wq")][:, kc, mc * 128:(mc + 1) * 128],
                                         xa[:, kc, :], start=(kc == 0), stop=(kc == CH - 1))
                    t1 = scrpool.tile([128, 512], dt.bfloat16, tag="scr",
                                      name=f"qc_{i}_{mc}")
                    nc.vector.scalar_tensor_tensor(
                        t1[:], pq[:], 0.0, cq[:, mc, :], AO.add, AO.mult)
                    qsh = scfpool.tile([128, 512], dt.float32, tag="ksh",
                                       name=f"qsh_{i}_{mc}")
                    nc.vector.stream_shuffle(qsh[:], pq[:], SWAP_MASK)
                    t2 = scrpool.tile([128, 512], dt.bfloat16, tag="scr",
                                      name=f"qs_{i}_{mc}")
                    nc.vector.scalar_tensor_tensor(
                        t2[:], qsh[:], 0.0, sq[:, mc, :], AO.add, AO.mult)
                    nc.gpsimd.tensor_tensor(qb[:, mc, :], t1[:], t2[:], AO.add)

                # ---- attention (+ next layer's K-proj interleaved) ----
                kb = kbs[i]
                o_raw = apool.tile([128, CH, NT], dt.bfloat16, tag="act",
                                   name=f"oraw_{i}")
                # pairs 0-2 and pair 3 get separate base-0 den/recip tiles:
                # group A's 1/den computes mid-attention, group B right at the
                # end — both ready before their rb matmuls in the post phase
                den2t = [smpool.tile([6, 512], dt.float32r, tag="den0", bufs=1,
                                     name=f"den0_{i}"),
                         smpool.tile([2, 512], dt.float32r, tag="den1", bufs=1,
                                     name=f"den1_{i}")]
                for p in range(H // 2):
                    hpair = (2 * p, 2 * p + 1)
                    if i == 0:
                        kv_k_chunk(0, p)
                    oacc = {h: ppool.tile([128, 512], dt.float32, tag="mm",
                                          name=f"oacc_{i}_{h}") for h in hpair}
                    Eg = {}

                    def attn_v(g):
                        for h in hpair:
                            for j in range(2):
                                nidx = g * 2 + j
                                nc.tensor.matmul(
                                    oacc[h][0:HD + 1, :], vb[:, nidx, h, :],
                                    Eg[(h, g)][:, j, :], start=(nidx == 0),
                                    stop=(nidx == 15), skip_group_check=True)

                    for g in range(8):
                        ps = {h: ppool.tile([128, 2, 512], dt.float32, tag="sc",
                                            name=f"ps_{i}_{h}_{g}") for h in hpair}
                        for j in range(2):
                            nidx = g * 2 + j
                            for h in hpair:
                                off = (h % 2) * 64
                                nc.tensor.matmul(
                                    ps[h][:, j, :],
                                    kb[off:off + 64, p, nidx * 128:(nidx + 1) * 128],
                                    qb[off:off + 64, p, :], start=True, stop=True)
                        for h in hpair:
                            E = epool.tile([128, 2, 512], dt.bfloat16, tag="E",
                                           name=f"E_{i}_{h}_{g}")
                            nc.scalar.activation(E[:], ps[h][:], AF.Exp)
                            Eg[(h, g)] = E
                        if g >= 2:
                            attn_v(g - 2)
                    attn_v(6)
                    attn_v(7)
                    half = 0 if p < 3 else 1
                    for h in hpair:
                        off = (h % 2) * 64
                        nc.vector.tensor_copy(o_raw[off:off + 64, p, :], oacc[h][0:HD, :])
                        dh = smpool.tile([1, 512], dt.float32r, tag="denh", bufs=1,
                                         name=f"dh_{i}_{h}")
                        nc.vector.tensor_copy(dh[:], oacc[h][HD:HD + 1, :])
                        row = (2 * p + h % 2) if p < 3 else (h % 2)
                        nc.sync.dma_start(den2t[half][row:row + 1, :], dh[:])
                    if p == 2 or p == 3:
                        # 1/den for this group, in place, inside the exp
                        # window (off the post critical path)
                        nc.scalar.activation(den2t[half][:], den2t[half][:], AF.Ln)
                        nc.scalar.activation(den2t[half][:], den2t[half][:],
                                             AF.Exp, scale=-1.0)
                    if i + 1 < L:
                        kv_k_chunk(i + 1, p)

                # ---- out projection; r1 = po*recip + q ; fused LN1 stats.
                # rb/finish lag one chunk behind po so the PE queue never
                # head-of-line blocks on the recip chain or evictions.
                r1 = rpool.tile([128, CH, NT], dt.float32, tag="resid", name=f"r1_{i}")
                pm1 = ppool.tile([128, 512], dt.float32, tag="mm", name=f"pm1_{i}")
                pv1 = ppool.tile([128, 512], dt.float32, tag="mm", name=f"pv1_{i}")
                store1 = {}
                po = {}

                def fin1(mo):
                    ind = indA if mo < 3 else indB
                    rb = ppool.tile([128, 512], dt.float32, tag="sc", name=f"rb_{i}_{mo}")
                    nc.tensor.matmul(rb[:], ind[:, mo * 128:(mo + 1) * 128],
                                     den2t[0 if mo < 3 else 1][:], start=True, stop=True)
                    rb_s = scrpool.tile([128, 512], dt.float32, tag="rbs", bufs=2,
                                        name=f"rbs_{i}_{mo}")
                    nc.scalar.copy(rb_s[:], rb[:])
                    t = scfpool.tile([128, 512], dt.float32, tag="ksh",
                                     name=f"rt_{i}_{mo}")
                    nc.vector.tensor_mul(t[:], po[mo][:], rb_s[:])
                    nc.gpsimd.tensor_tensor(r1[:, mo, :], t[:], q[:, mo, :], AO.add)
                    _ln_chunk_evict(i, 0, r1, mo, store1)
                    if mo >= 1:
                        _ln_stats_mm(store1, mo - 1, pm1, pv1)

                for mo in range(CH):
                    po[mo] = ppool.tile([128, 512], dt.float32, tag="kvmm",
                                        name=f"po_{i}_{mo}")
                    for kc in range(CH):
                        nc.tensor.matmul(po[mo][:], wq[(i, "wo")][:, kc, mo * 128:(mo + 1) * 128],
                                         o_raw[:, kc, :], start=(kc == 0), stop=(kc == CH - 1))
                    if mo >= 1:
                        fin1(mo - 1)
                fin1(CH - 1)
                _ln_stats_mm(store1, CH - 1, pm1, pv1)
                ln1 = _ln_head(i, 0, pm1, pv1)

                # ---- next layer's V projection, first half (PE slack while
                # the LN1 stats chain resolves) ----
                if i + 1 < L:
                    kv_v(i + 1, range(0, 8))

                # ---- LN1 -> q, with per-chunk adaln(ffn) follow-on ----
                xfb = apool.tile([128, CH, NT], dt.bfloat16, tag="act", name=f"xfb_{i}")

                def emit_xfb(c):
                    nc.vector.tensor_scalar(
                        xfb[:, c, :], q[:, c, :],
                        ada[:, ada_col(i, 2, c):ada_col(i, 2, c) + 1],
                        ada[:, ada_col(i, 3, c):ada_col(i, 3, c) + 1],
                        AO.mult, AO.add)

                _ln_apply(i, 0, r1, *ln1, after_chunk=emit_xfb)

                # ---- FFN ; fused LN2 stats ----
                hbf = apool.tile([128, CH, NT], dt.bfloat16, tag="act", name=f"hbf_{i}")
                for mh in range(CH):
                    ph = ppool.tile([128, 512], dt.float32, tag="kvmm", name=f"ph_{i}_{mh}")
                    for kc in range(CH):
                        nc.tensor.matmul(ph[:], wq[(i, "w1")][:, kc, mh * 128:(mh + 1) * 128],
                                         xfb[:, kc, :], start=(kc == 0), stop=(kc == CH - 1))
                    nc.vector.tensor_scalar_max(hbf[:, mh, :], ph[:], 0.0)
                r2 = rpool.tile([128, CH, NT], dt.float32, tag="resid", name=f"r2_{i}")
                pm2 = ppool.tile([128, 512], dt.float32, tag="mm", name=f"pm2_{i}")
                pv2 = ppool.tile([128, 512], dt.float32, tag="mm", name=f"pv2_{i}")
                store2 = {}
                pf = {}

                def fin2(mo):
                    nc.vector.tensor_add(r2[:, mo, :], pf[mo][:], xfb[:, mo, :])
                    _ln_chunk_evict(i, 1, r2, mo, store2)
                    if mo >= 1:
                        _ln_stats_mm(store2, mo - 1, pm2, pv2)

                for mo in range(CH):
                    pf[mo] = ppool.tile([128, 512], dt.float32, tag="kvmm",
                                        name=f"pf_{i}_{mo}")
                    for kc in range(CH):
                        nc.tensor.matmul(pf[mo][:], wq[(i, "w2")][:, kc, mo * 128:(mo + 1) * 128],
                                         hbf[:, kc, :], start=(kc == 0), stop=(kc == CH - 1))
                    if mo >= 1:
                        fin2(mo - 1)
                fin2(CH - 1)
                _ln_stats_mm(store2, CH - 1, pm2, pv2)
                ln2 = _ln_head(i, 1, pm2, pv2)

                # ---- next layer's V projection, second half (PE slack while
                # the LN2 stats chain resolves) ----
                if i + 1 < L:
                    kv_v(i + 1, range(8, 16))

                # ---- LN2 -> q ; emit layer output per chunk ----
                _ln_apply(i, 1, r2, *ln2,
                          after_chunk=lambda c: nc.sync.dma_start(d_out[i, :, c, :], q[:, c, :]))

            def _ln_chunk_evict(i, which, rin, c, store):
                # bf16 copy + square right after rin chunk lands (DVE + Pool)
                rbf = scrpool.tile([128, 512], dt.bfloat16, tag="scr",
                                   name=f"rbf_{i}_{which}_{c}")
                nc.vector.tensor_copy(rbf[:], rin[:, c, :])
                r2b = scrpool.tile([128, 512], dt.bfloat16, tag="scr",
                                   name=f"r2b_{i}_{which}_{c}")
                nc.vector.tensor_mul(r2b[:], rbf[:], rbf[:])
                store[c] = (rbf, r2b)

            def _ln_stats_mm(store, c, pm, pv):
                # lag-1 accumulation so the PE queue rarely waits on evicts
                rbf, r2b = store[c]
                nc.tensor.matmul(pm[0:1, :], ones_sc[:], rbf[:],
                                 start=(c == 0), stop=(c == CH - 1),
                                 skip_group_check=True)
                nc.tensor.matmul(pv[0:1, :], ones_sc[:], r2b[:],
                                 start=(c == 0), stop=(c == CH - 1),
                                 skip_group_check=True)

            def _ln_head(i, which, pm, pv):
                # stat chain emitted FIRST so the DVE/ACT queue heads aren't
                # blocked behind filler work
                m_sb = stpool.tile([1, 512], dt.float32r, tag="st", name=f"m_{i}_{which}")
                nc.vector.tensor_copy(m_sb[:], pm[0:1, :])
                msq = stpool.tile([1, 512], dt.float32, tag="st", name=f"msq_{i}_{which}")
                nc.scalar.activation(msq[:], pm[0:1, :], AF.Square)
                var = stpool.tile([1, 512], dt.float32, tag="st", name=f"var_{i}_{which}")
                nc.vector.scalar_tensor_tensor(var[:], pv[0:1, :], 1e-5, msq[:],
                                               AO.add, AO.subtract)
                lnv = stpool.tile([1, 512], dt.float32, tag="st", name=f"lnv_{i}_{which}")
                nc.scalar.activation(lnv[:], var[:], AF.Ln)
                rstd = stpool.tile([1, 512], dt.float32r, tag="st", name=f"rstd_{i}_{which}")
                nc.scalar.activation(rstd[:], lnv[:], AF.Exp, scale=-0.5)
                return m_sb, rstd

            def _ln_apply(i, which, rin, m_sb, rstd, after_chunk=None):
                mb = ppool.tile([128, 512], dt.float32, tag="sc", name=f"mb_{i}_{which}")
                nc.tensor.matmul(mb[:], ones1[:], m_sb[:], start=True, stop=True)
                rsb = ppool.tile([128, 512], dt.float32, tag="sc", name=f"rsb_{i}_{which}")
                nc.tensor.matmul(rsb[:], ones1[:], rstd[:], start=True, stop=True)
                for c in range(CH):
                    t1 = scfpool.tile([128, 512], dt.float32, tag="scf",
                                      name=f"lt_{i}_{which}_{c}")
                    nc.vector.scalar_tensor_tensor(t1[:], rin[:, c, :], 0.0, mb[:],
                                                   AO.add, AO.subtract)
                    nc.vector.tensor_mul(q[:, c, :], t1[:], rsb[:])
                    if after_chunk is not None:
                        after_chunk(c)

            def body(first):
                # one shared kb: layer i+1's chunk-p write WAR-syncs against
                # layer i's chunk-p scores via subtile deps
                kb1 = spool.tile([128, CH, NK], dt.bfloat16, tag="kb", name="kb1")
                for i in range(L):
                    kbs[i] = kb1
                load_wkv(0)
                load_wkv(1)
                load_wq(0)
                if first:
                    load_late_consts()
                kv_v(0, range(16))
                for i in range(L):
                    layer(i)
                kbs.clear()
                wkv.clear()
                wq.clear()

            if nrep == 1:
                body(first=True)
            else:
                load_late_consts()
                with tc.For_i(0, nrep, 1):
                    for _u in range(unroll):
                        body(first=False)

    nc.compile()
    return nc


def _prep_core(inputs, core, host):
    b, qh = core // 2, core % 2
    sl = slice(qh * NT, (qh + 1) * NT)
    im = {
        "qT": _pack_fm(inputs["query"][sl, b, :].T).astype(BF),
        "vT8": host["v8"][b],
        "cq": _pack_fm(inputs["query_pos"][b, sl, :, 0].T).astype(BF),
        "sq": _pack_fm(host["sgn"] * inputs["query_pos"][b, sl, :, 1].T).astype(BF),
        "ck": _pack_fm(inputs["value_pos"][b, :, :, 0].T / W8SCALE).astype(BF),
        "sk": _pack_fm(inputs["value_pos"][b, :, :, 1].T / W8SCALE).astype(BF),
        "w8": host["w8"],
        "wts": host["wts"],
        "ada": host["ada"][b],
        "ind8": host["ind8"],
        "ones1": np.ones((1, 128), np.float32),
    }
    return im


def _prep_host(inputs):
    import concourse.mybir as mybir
    F8 = mybir.dt.np(mybir.dt.float8e4)

    wts = np.zeros((L, 4, 128, CH, C), BF)
    w8 = np.zeros((L, 3, 128, 2, 2, C), F8)
    for i in range(L):
        in_w, in_b = np.asarray(inputs["in_w"][i]), np.asarray(inputs["in_b"][i])
        wq = in_w[:C] * SCALING
        wk, wv = in_w[C:2 * C], in_w[2 * C:]
        if np.any(in_b):
            raise NotImplementedError("nonzero in-projection bias not supported")
        w8[i, 0] = _pack_w8(np.ascontiguousarray(wk.T) * W8SCALE).astype(F8)
        w8[i, 1] = _pack_w8(np.ascontiguousarray(_rot2_rows(wk).T) * W8SCALE).astype(F8)
        w8[i, 2] = _pack_w8(np.ascontiguousarray(wv.T) * W8SCALE).astype(F8)
        mats = [wq.T, np.asarray(inputs["out_w"][i]).T,
                np.asarray(inputs["w1"][i]).T, np.asarray(inputs["w2"][i]).T]
        for j, m in enumerate(mats):
            wts[i, j] = _pack_w(np.ascontiguousarray(m)).astype(BF)

    v8 = np.zeros((4, 128, 2, 2, NK), F8)
    for b in range(4):
        v8[b] = _pack_v8(np.asarray(inputs["value"][:, b, :], np.float32)).astype(F8)

    ada = np.zeros((4, 128, L * 4 * CH), np.float32)
    diff = np.asarray(inputs["diff_ts"], np.float32)
    for b in range(4):
        st = _silu(diff[b])
        for i in range(L):
            for qy, (aw, ab) in enumerate(
                    [(inputs["aw_attn"][i], inputs["ab_attn"][i]),
                     (inputs["aw_ffn"][i], inputs["ab_ffn"][i])]):
                mod = st @ np.asarray(aw, np.float32).T + np.asarray(ab, np.float32)
                sc, sh = 1.0 + mod[:C], mod[C:]
                for c in range(CH):
                    ada[b, :, (i * 4 + 2 * qy) * CH + c] = sc[c * 128:(c + 1) * 128]
                    ada[b, :, (i * 4 + 2 * qy + 1) * CH + c] = sh[c * 128:(c + 1) * 128]

    ind8 = np.zeros((8, C), np.float32)
    for h in range(H):
        base = (h // 2) * 128 + (h % 2) * 64
        ind8[h, base:base + 64] = 1.0
    sgn = np.ones((C, 1), np.float32)
    sgn[0::2] = -1.0

    flags = (bool(np.any(np.asarray(inputs["in_b"]))),
             bool(np.any(np.asarray(inputs["out_b"]))),
             bool(np.any(np.asarray(inputs["b1"]))),
             bool(np.any(np.asarray(inputs["b2"]))),
             bool(np.any(np.asarray(inputs["ln1_g"]) != 1.0) or np.any(np.asarray(inputs["ln2_g"]) != 1.0)),
             bool(np.any(np.asarray(inputs["ln1_b"])) or np.any(np.asarray(inputs["ln2_b"]))))
    return dict(wts=wts, w8=w8, v8=v8, ada=ada, ind8=ind8, sgn=sgn), flags


def _get_program(flags, nrep=1, unroll=1):
    key = (flags, nrep, unroll)
    if key not in _CACHE:
        _CACHE[key] = _build(flags, nrep, unroll)
    return _CACHE[key]


def _assemble(results):
    full = np.zeros((L, 1024, 4, C), np.float32)
    for core in range(8):
        b, qh = core // 2, core % 2
        arr = np.asarray(results[core]["out"], np.float32)   # [L, 128, CH, NT]
        fm = np.transpose(arr, (0, 2, 1, 3)).reshape(L, C, NT)
        full[:, qh * NT:(qh + 1) * NT, b, :] = np.transpose(fm, (0, 2, 1))
    return full


def kernel(**inputs):
    from concourse.bass_utils import run_bass_kernel_spmd

    inputs = {k: np.asarray(v) for k, v in inputs.items()}
    host, flags = _prep_host(inputs)
    nc = _get_program(flags)
    in_maps = [_prep_core(inputs, core, host) for core in range(8)]
    res = run_bass_kernel_spmd(nc, in_maps, list(range(8)))
    return _assemble(res.results)
